# revision 27
# baseline (speedup 1.0000x reference)
"""Multi-head attention (B=2, L=2048, H=16, D=64) on 8 TRN2 NeuronCores.

Sharding: core = (batch b, head-group hg); 2 batches x 4 groups of 4 heads.
Per core, for its batch and its 4 heads (2 head-pairs):
    Q^T/K^T = W^T x^T           (d on partitions; pair m: head 2m at rows
                                 0:64, head 2m+1 at rows 64:128)
    V       = x W_v             (j on partitions, + ones column for denom)
    S^T     = K^T.T Q^T         (j on partitions, i free)
    P'      = exp(S^T/8)        (un-normalized softmax numerator)
    O'^T    = [V|1].T P'        (row 64 = softmax denominator)
    O^T     = O'[0:64] * (1/O'[64])
    out^T  += Wo_rows^T O^T     (partial over head-group rows of Wo)
Host sums the 4 partials per batch, transposes, adds bo.

All matmuls run in float32r (TF32-like; full PE rate at N>=256; inputs
rounded by the PE itself when DRAM tensors are declared float32r).

Schedule notes (from perfetto traces):
  - attention runs ~1.25us per [128,1024] exp tile, PE and ACT co-bound.
  - softmax denominators: reciprocal is computed at full DVE width by
    DMA-reshaping the [1,1024] denominator row to [128,8]; the result is
    broadcast across partitions on the idle GPSIMD engine
    (partition_broadcast requires a physical partition-0 source).
  - each unit's normalization is deferred into the middle of the NEXT
    unit's j-loop so its latency hides under ~18us of PE work.
  - PSUM is fully subscribed in steady state (2x S ping-pong + 2 head
    accumulators = 8 banks), which forces projections and the output
    projection into serial phases around the attention loop.
"""

import sys

try:
    import concourse.bass as bass  # noqa: F401
except ImportError:  # pragma: no cover - path fallback
    sys.path.insert(0, "/opt/trn_rl_repo")

import numpy as np
import concourse.bass as bass
import concourse.mybir as mybir
import concourse.tile as tile
from concourse import bacc
from concourse.bass_utils import run_bass_kernel_spmd

F32 = mybir.dt.float32
F32R = mybir.dt.float32r
AF = mybir.ActivationFunctionType

B = 2
L = 2048          # sequence length
C = 1024          # model dim
H_LOC = 4         # heads per core
D = 64            # head dim
HD = H_LOC * D    # 256 = local head-group width
KT = C // 128     # 8 k-tiles over the model dim
SCALE2 = float(D) ** -0.5  # 1/8, applied once inside exp

_cache = {}


def _build():
    nc = bacc.Bacc("TRN2", target_bir_lowering=False, debug=False, num_devices=8)

    xT = nc.declare_dram_parameter("xT", [C, L], F32R, isOutput=False)
    wq = nc.declare_dram_parameter("wq", [C, HD], F32R, isOutput=False)
    wk = nc.declare_dram_parameter("wk", [C, HD], F32R, isOutput=False)
    wv = nc.declare_dram_parameter("wv", [C, HD], F32R, isOutput=False)
    wo = nc.declare_dram_parameter("wo", [HD, C], F32R, isOutput=False)
    outT = nc.declare_dram_parameter("outT", [C, L], F32, isOutput=True)

    with tile.TileContext(nc) as tc:
        with tc.tile_pool(name="sb", bufs=1) as sb, \
             tc.tile_pool(name="ps", bufs=2, space="PSUM") as ps, \
             tc.tile_pool(name="po", bufs=2, space="PSUM") as po:

            sbx = tc.alloc_tile_pool(name="sbx", bufs=1)

            # ---- load inputs (wq/wk first so projections start early) ------
            wq_sb = sb.tile([128, KT, HD], F32R, tag="wq")
            wk_sb = sb.tile([128, KT, HD], F32R, tag="wk")
            wv_sb = sb.tile([128, KT, HD], F32R, tag="wv")
            xT_sb = sbx.tile([128, KT, L], F32R, tag="xT")
            # first k-tile of wq + xT land first so the k=0 matmul starts early
            nc.sync.dma_start(wq_sb[:, 0, :], wq[0:128, :])
            nc.sync.dma_start(xT_sb[:, 0, :], xT[0:128, :])
            nc.sync.dma_start(
                wq_sb[:, 1:, :],
                wq[128:, :].rearrange("(k p) c -> p k c", p=128))
            nc.sync.dma_start(wk_sb[:, :, :], wk.rearrange("(k p) c -> p k c", p=128))
            for k in range(1, KT):
                nc.sync.dma_start(xT_sb[:, k, :], xT[k * 128:(k + 1) * 128, :])

            nc.sync.dma_start(wv_sb[:, :, :], wv.rearrange("(k p) c -> p k c", p=128))
            wo_sb = sb.tile([128, 2, C], F32R, tag="wo")
            nc.sync.dma_start(wo_sb[:, :, :], wo.rearrange("(k p) c -> p k c", p=128))

            ones_f = sb.tile([128, 64], F32, tag="ones_f")
            nc.vector.memset(ones_f[:], 1.0)

            # ---- projections ------------------------------------------------
            qT_sb = sb.tile([128, 2, L], F32R, tag="qT")
            kT_sb = sb.tile([128, 2, L], F32R, tag="kT")

            def emit_qk_proj(m):
                for w_sb, t_sb in ((wq_sb, qT_sb), (wk_sb, kT_sb)):
                    for n in range(4):
                        p = ps.tile([128, 1024], F32, tag="s")
                        acc = p[:, 0:512]
                        for k in range(KT):
                            nc.tensor.matmul(
                                acc,
                                w_sb[:, k, m * 128:(m + 1) * 128],
                                xT_sb[:, k, n * 512:(n + 1) * 512],
                                start=(k == 0), stop=(k == KT - 1),
                            )
                        nc.vector.tensor_copy(
                            t_sb[:, m, n * 512:(n + 1) * 512], acc)

            emit_qk_proj(0)

            # V with ones column: v_sb[p, j_tile, h, 0:64]=V, [..., 64]=1
            v_sb = sb.tile([128, 16, H_LOC, D + 1], F32R, tag="v")
            nc.vector.tensor_copy(
                v_sb[:, :, :, D:D + 1],
                ones_f.rearrange("p (a b c) -> p a b c", a=16, b=4),
            )
            for it in range(16):
                p = po.tile([128, 1024], F32, tag="o")
                acc = p[:, 0:HD]
                for k in range(KT):
                    nc.tensor.matmul(
                        acc,
                        xT_sb[:, k, it * 128:(it + 1) * 128],
                        wv_sb[:, k, :],
                        start=(k == 0), stop=(k == KT - 1),
                    )
                nc.vector.tensor_copy(
                    v_sb[:, it, :, 0:D],
                    acc.rearrange("p (h d) -> p h d", h=H_LOC),
                )

            emit_qk_proj(1)

            # x^T no longer needed; release its SBUF for the attention pools
            sbx.release()

            es_pool = tc.alloc_tile_pool(name="es_pool", bufs=6)
            st_pool = tc.alloc_tile_pool(name="st_pool", bufs=2)
            ost_pool = tc.alloc_tile_pool(name="ost_pool", bufs=6)
            np_pool = tc.alloc_tile_pool(name="np_pool", bufs=4)
            d0_pool = tc.alloc_tile_pool(name="d0_pool", bufs=1)

            # ---- attention + interleaved output projection ------------------
            oT_sb = sb.tile([128, 2, L], F32R, tag="oT")

            pending = []   # deferred normalize: (m, i0, [o_cp x2])
            wo_queue = []  # (ih, ct, n) output-projection chunks to interleave

            def emit_normalize():
                m, i0, o_cps, d0s = pending.pop(0)
                for hl in range(2):
                    rep_sb = st_pool.tile([64, 1024], F32R, tag="rep")
                    nc.gpsimd.partition_broadcast(rep_sb[:], d0s[hl][:])
                    with nc.allow_low_precision(reason="f32r matmul input"):
                        if hl == 0:
                            nc.vector.tensor_mul(
                                oT_sb[0:64, m, i0:i0 + 1024],
                                o_cps[hl][0:64, :], rep_sb[:])
                        else:
                            stage = st_pool.tile([64, 1024], F32R, tag="stage")
                            nc.vector.tensor_mul(
                                stage[:], o_cps[hl][0:64, :], rep_sb[:])
                            nc.gpsimd.dma_start(
                                oT_sb[64:128, m, i0:i0 + 1024], stage[:])

            def emit_wo_chunk(ih, ct):
                # [128, 1024] output chunk; kk-outer reuses each Wo k-tile's
                # LDWEIGHTS across both 512-wide matmuls
                i0 = ih * 1024
                acc = ps.tile([128, 1024], F32, tag="s", name="wo_ps")
                for kk in range(2):
                    for n in range(2):
                        nc.tensor.matmul(
                            acc[:, n * 512:(n + 1) * 512],
                            wo_sb[:, kk, ct * 128:(ct + 1) * 128],
                            oT_sb[:, kk, i0 + n * 512:i0 + (n + 1) * 512],
                            start=(kk == 0), stop=(kk == 1),
                        )
                ost = ost_pool.tile([128, 1024], F32, tag="ost", name="ost")
                nc.vector.tensor_copy(ost[:], acc[:])
                nc.sync.dma_start(
                    outT[ct * 128:(ct + 1) * 128, i0:i0 + 1024], ost[:])

            units = [(ih, m) for ih in range(2) for m in range(2)]
            for ui, (ih, m) in enumerate(units):
                i0 = ih * 1024
                o_h = []
                for hl in range(2):
                    of = po.tile([128, 1024], F32, tag="o", name=f"o_ps{hl}")
                    o_h.append(of[0:65, :])
                es = [None, None]
                for j in range(16):
                    if j == 8 and pending:
                        emit_normalize()  # prior unit; reciprocal done by now
                    s_list = []
                    for hl in range(2):
                        r0 = hl * 64
                        s_ps = ps.tile([128, 1024], F32, tag="s", name=f"s_ps{hl}")
                        for n in range(2):
                            nc.tensor.matmul(
                                s_ps[:, n * 512:(n + 1) * 512],
                                kT_sb[r0:r0 + 64, m, j * 128:(j + 1) * 128],
                                qT_sb[r0:r0 + 64, m,
                                      i0 + n * 512:i0 + (n + 1) * 512],
                                start=True, stop=True,
                            )
                        s_list.append(s_ps)
                    prev_es = es
                    es = []
                    for hl in range(2):
                        e_sb = es_pool.tile([128, 1024], F32R, tag="es",
                                            name=f"es{hl}")
                        nc.scalar.activation(e_sb[:], s_list[hl][:], AF.Exp,
                                             scale=SCALE2)
                        es.append(e_sb)
                    # AV for step j-1 (software-pipelined one step behind)
                    if j > 0:
                        for hl in range(2):
                            for n in range(2):
                                nc.tensor.matmul(
                                    o_h[hl][:, n * 512:(n + 1) * 512],
                                    v_sb[:, j - 1, 2 * m + hl, :],
                                    prev_es[hl][:, n * 512:(n + 1) * 512],
                                    start=(j == 1), stop=False,
                                )
                # epilogue AV for j=15
                for hl in range(2):
                    for n in range(2):
                        nc.tensor.matmul(
                            o_h[hl][:, n * 512:(n + 1) * 512],
                            v_sb[:, 15, 2 * m + hl, :],
                            es[hl][:, n * 512:(n + 1) * 512],
                            start=False, stop=True,
                        )
                # pull O' off PSUM (rounded to f32r), reciprocal in place on
                # the denominator row, defer the PE-side normalize
                o_cps, d0s = [], []
                for hl in range(2):
                    o_cp = np_pool.tile([65, 1024], F32R, tag="o_cp",
                                        name=f"o_cp{hl}")
                    with nc.allow_low_precision(reason="f32r matmul input"):
                        nc.vector.tensor_copy(o_cp[:], o_h[hl][:])
                    # reshape the denominator row across all 128 lanes so the
                    # reciprocal runs at full DVE width, then reshape back
                    dsq = d0_pool.tile([128, 8], F32R, tag=f"dsq_{hl}")
                    nc.gpsimd.dma_start(dsq[:], o_cp[64:65, :])
                    with nc.allow_low_precision(reason="f32r matmul input"):
                        nc.vector.reciprocal(dsq[:], dsq[:])
                    d0 = d0_pool.tile([1, 1024], F32R, tag=f"d0_{hl}")
                    nc.gpsimd.dma_start(d0[:], dsq[:])
                    o_cps.append(o_cp)
                    d0s.append(d0)
                pending.append((m, i0, o_cps, d0s))

            for ct in range(8):
                emit_wo_chunk(0, ct)
            while pending:
                emit_normalize()
            for ct in range(8):
                emit_wo_chunk(1, ct)

            d0_pool.release()
            np_pool.release()
            ost_pool.release()
            st_pool.release()
            es_pool.release()

    nc.compile()
    return nc


def kernel(x, Wq, Wk, Wv, Wo, bo):
    x = np.asarray(x, dtype=np.float32)
    Wq = np.asarray(Wq, dtype=np.float32)
    Wk = np.asarray(Wk, dtype=np.float32)
    Wv = np.asarray(Wv, dtype=np.float32)
    Wo = np.asarray(Wo, dtype=np.float32)
    bo = np.asarray(bo, dtype=np.float32)

    if "nc" not in _cache:
        _cache["nc"] = _build()
    nc = _cache["nc"]

    xTs = [np.ascontiguousarray(x[b].T) for b in range(B)]
    in_maps = []
    for core in range(8):
        b, hg = divmod(core, 4)
        sl = slice(hg * HD, (hg + 1) * HD)
        in_maps.append({
            "xT": xTs[b],
            "wq": np.ascontiguousarray(Wq[:, sl]),
            "wk": np.ascontiguousarray(Wk[:, sl]),
            "wv": np.ascontiguousarray(Wv[:, sl]),
            "wo": np.ascontiguousarray(Wo[sl, :]),
        })

    res = run_bass_kernel_spmd(nc, in_maps, core_ids=list(range(8)))
    out = np.empty((B, L, C), dtype=np.float32)
    for b in range(B):
        acc = res.results[4 * b]["outT"]
        for hg in range(1, 4):
            acc = acc + res.results[4 * b + hg]["outT"]
        out[b] = acc.T + bo
    return out


# revision 28
# speedup vs baseline: 1.4382x; 1.4382x over previous
"""Multi-head attention (B=2, L=2048, H=16, D=64) on 8 TRN2 NeuronCores.

Sharding: core = (batch b, head-group hg); 2 batches x 4 groups of 4 heads.
Per core, for its batch and its 4 heads (2 head-pairs):
    Q^T/K^T = W^T x^T           (d on partitions; pair m: head 2m at rows
                                 0:64, head 2m+1 at rows 64:128)
    V       = x W_v             (j on partitions, + ones column for denom)
    S^T     = K^T.T Q^T         (j on partitions, i free)
    P'      = exp(S^T/8)        (un-normalized softmax numerator)
    O'^T    = [V|1].T P'        (row 64 = softmax denominator)
    O^T     = O'[0:64] * (1/O'[64])
    out^T  += Wo_rows^T O^T     (partial over head-group rows of Wo)
Host sums the 4 partials per batch, transposes, adds bo.

All matmuls run in float32r (TF32-like; full PE rate at N>=256; inputs
rounded by the PE itself when DRAM tensors are declared float32r).

Schedule notes (from perfetto traces):
  - attention runs ~1.25us per [128,1024] exp tile, PE and ACT co-bound.
  - softmax denominators: reciprocal is computed at full DVE width by
    DMA-reshaping the [1,1024] denominator row to [128,8]; the result is
    broadcast across partitions on the idle GPSIMD engine
    (partition_broadcast requires a physical partition-0 source).
  - each unit's normalization is deferred into the middle of the NEXT
    unit's j-loop so its latency hides under ~18us of PE work.
  - PSUM is fully subscribed in steady state (2x S ping-pong + 2 head
    accumulators = 8 banks), which forces projections and the output
    projection into serial phases around the attention loop.
"""

import sys

try:
    import concourse.bass as bass  # noqa: F401
except ImportError:  # pragma: no cover - path fallback
    sys.path.insert(0, "/opt/trn_rl_repo")

import numpy as np
import concourse.bass as bass
import concourse.mybir as mybir
import concourse.tile as tile
from concourse import bacc
from concourse.bass_utils import run_bass_kernel_spmd

F32 = mybir.dt.float32
F32R = mybir.dt.float32r
AF = mybir.ActivationFunctionType

B = 2
L = 2048          # sequence length
C = 1024          # model dim
H_LOC = 4         # heads per core
D = 64            # head dim
HD = H_LOC * D    # 256 = local head-group width
KT = C // 128     # 8 k-tiles over the model dim
SCALE2 = float(D) ** -0.5  # 1/8, applied once inside exp

_cache = {}


def _build():
    nc = bacc.Bacc("TRN2", target_bir_lowering=False, debug=False, num_devices=8)

    xT = nc.declare_dram_parameter("xT", [C, L], F32R, isOutput=False)
    wq = nc.declare_dram_parameter("wq", [C, HD], F32R, isOutput=False)
    wk = nc.declare_dram_parameter("wk", [C, HD], F32R, isOutput=False)
    wv = nc.declare_dram_parameter("wv", [C, HD], F32R, isOutput=False)
    wo = nc.declare_dram_parameter("wo", [HD, C], F32R, isOutput=False)
    outT = nc.declare_dram_parameter("outT", [C, L], F32, isOutput=True)

    with tile.TileContext(nc) as tc:
        with tc.tile_pool(name="sb", bufs=1) as sb, \
             tc.tile_pool(name="ps", bufs=2, space="PSUM") as ps, \
             tc.tile_pool(name="po", bufs=2, space="PSUM") as po:

            sbx = tc.alloc_tile_pool(name="sbx", bufs=1)

            # ---- load inputs (wq/wk first so projections start early) ------
            wq_sb = sb.tile([128, KT, HD], F32R, tag="wq")
            wk_sb = sb.tile([128, KT, HD], F32R, tag="wk")
            wv_sb = sb.tile([128, KT, HD], F32R, tag="wv")
            nc.sync.dma_start(wq_sb[:, :, :], wq.rearrange("(k p) c -> p k c", p=128))
            xT_sb = sbx.tile([128, KT, L], F32R, tag="xT")
            nc.sync.dma_start(xT_sb[:, 0, :], xT[0:128, :])
            nc.sync.dma_start(wk_sb[:, :, :], wk.rearrange("(k p) c -> p k c", p=128))
            for k in range(1, KT):
                nc.sync.dma_start(xT_sb[:, k, :], xT[k * 128:(k + 1) * 128, :])

            nc.sync.dma_start(wv_sb[:, :, :], wv.rearrange("(k p) c -> p k c", p=128))
            wo_sb = sb.tile([128, 2, C], F32R, tag="wo")
            nc.sync.dma_start(wo_sb[:, :, :], wo.rearrange("(k p) c -> p k c", p=128))

            ones_f = sb.tile([128, 64], F32, tag="ones_f")
            nc.vector.memset(ones_f[:], 1.0)

            # ---- projections ------------------------------------------------
            qT_sb = sb.tile([128, 2, L], F32R, tag="qT")
            kT_sb = sb.tile([128, 2, L], F32R, tag="kT")

            def emit_qk_proj(m):
                for w_sb, t_sb in ((wq_sb, qT_sb), (wk_sb, kT_sb)):
                    for n in range(4):
                        p = ps.tile([128, 1024], F32, tag="s")
                        acc = p[:, 0:512]
                        for k in range(KT):
                            nc.tensor.matmul(
                                acc,
                                w_sb[:, k, m * 128:(m + 1) * 128],
                                xT_sb[:, k, n * 512:(n + 1) * 512],
                                start=(k == 0), stop=(k == KT - 1),
                            )
                        nc.vector.tensor_copy(
                            t_sb[:, m, n * 512:(n + 1) * 512], acc)

            emit_qk_proj(0)

            # V with ones column: v_sb[p, j_tile, h, 0:64]=V, [..., 64]=1
            v_sb = sb.tile([128, 16, H_LOC, D + 1], F32R, tag="v")
            nc.vector.tensor_copy(
                v_sb[:, :, :, D:D + 1],
                ones_f.rearrange("p (a b c) -> p a b c", a=16, b=4),
            )
            for it in range(16):
                p = po.tile([128, 1024], F32, tag="o")
                acc = p[:, 0:HD]
                for k in range(KT):
                    nc.tensor.matmul(
                        acc,
                        xT_sb[:, k, it * 128:(it + 1) * 128],
                        wv_sb[:, k, :],
                        start=(k == 0), stop=(k == KT - 1),
                    )
                nc.vector.tensor_copy(
                    v_sb[:, it, :, 0:D],
                    acc.rearrange("p (h d) -> p h d", h=H_LOC),
                )

            emit_qk_proj(1)

            # x^T no longer needed; release its SBUF for the attention pools
            sbx.release()

            es_pool = tc.alloc_tile_pool(name="es_pool", bufs=6)
            st_pool = tc.alloc_tile_pool(name="st_pool", bufs=2)
            ost_pool = tc.alloc_tile_pool(name="ost_pool", bufs=6)
            np_pool = tc.alloc_tile_pool(name="np_pool", bufs=3)
            d0_pool = tc.alloc_tile_pool(name="d0_pool", bufs=1)

            # ---- attention + interleaved output projection ------------------
            oT_sb = sb.tile([128, 2, L], F32R, tag="oT")

            pending = []   # deferred normalize: (m, i0, [o_cp x2])
            wo_queue = []  # (ih, ct, n) output-projection chunks to interleave

            def emit_normalize():
                m, i0, o_cps, d0s = pending.pop(0)
                for hl in range(2):
                    rep_sb = st_pool.tile([64, 1024], F32R, tag="rep")
                    nc.gpsimd.partition_broadcast(rep_sb[:], d0s[hl][:])
                    with nc.allow_low_precision(reason="f32r matmul input"):
                        if hl == 0:
                            nc.vector.tensor_mul(
                                oT_sb[0:64, m, i0:i0 + 1024],
                                o_cps[hl][0:64, :], rep_sb[:])
                        else:
                            stage = st_pool.tile([64, 1024], F32R, tag="stage")
                            nc.vector.tensor_mul(
                                stage[:], o_cps[hl][0:64, :], rep_sb[:])
                            nc.gpsimd.dma_start(
                                oT_sb[64:128, m, i0:i0 + 1024], stage[:])

            def emit_wo_chunk(ih, ct):
                # [128, 1024] output chunk; kk-outer reuses each Wo k-tile's
                # LDWEIGHTS across both 512-wide matmuls
                i0 = ih * 1024
                acc = ps.tile([128, 1024], F32, tag="s", name="wo_ps")
                for kk in range(2):
                    for n in range(2):
                        nc.tensor.matmul(
                            acc[:, n * 512:(n + 1) * 512],
                            wo_sb[:, kk, ct * 128:(ct + 1) * 128],
                            oT_sb[:, kk, i0 + n * 512:i0 + (n + 1) * 512],
                            start=(kk == 0), stop=(kk == 1),
                        )
                ost = ost_pool.tile([128, 1024], F32, tag="ost", name="ost")
                nc.vector.tensor_copy(ost[:], acc[:])
                nc.sync.dma_start(
                    outT[ct * 128:(ct + 1) * 128, i0:i0 + 1024], ost[:])

            units = [(ih, m) for ih in range(2) for m in range(2)]
            for ui, (ih, m) in enumerate(units):
                i0 = ih * 1024
                o_h = []
                for hl in range(2):
                    of = po.tile([128, 1024], F32, tag="o", name=f"o_ps{hl}")
                    o_h.append(of[0:65, :])
                es = [None, None]
                for j in range(16):
                    if j == 8 and pending:
                        emit_normalize()  # prior unit; reciprocal done by now
                    s_list = []
                    for hl in range(2):
                        r0 = hl * 64
                        s_ps = ps.tile([128, 1024], F32, tag="s", name=f"s_ps{hl}")
                        for n in range(2):
                            nc.tensor.matmul(
                                s_ps[:, n * 512:(n + 1) * 512],
                                kT_sb[r0:r0 + 64, m, j * 128:(j + 1) * 128],
                                qT_sb[r0:r0 + 64, m,
                                      i0 + n * 512:i0 + (n + 1) * 512],
                                start=True, stop=True,
                            )
                        s_list.append(s_ps)
                    prev_es = es
                    es = []
                    for hl in range(2):
                        e_sb = es_pool.tile([128, 1024], F32R, tag="es",
                                            name=f"es{hl}")
                        nc.scalar.activation(e_sb[:], s_list[hl][:], AF.Exp,
                                             scale=SCALE2)
                        es.append(e_sb)
                    # AV for step j-1 (software-pipelined one step behind)
                    if j > 0:
                        for hl in range(2):
                            for n in range(2):
                                nc.tensor.matmul(
                                    o_h[hl][:, n * 512:(n + 1) * 512],
                                    v_sb[:, j - 1, 2 * m + hl, :],
                                    prev_es[hl][:, n * 512:(n + 1) * 512],
                                    start=(j == 1), stop=False,
                                )
                # epilogue AV for j=15
                for hl in range(2):
                    for n in range(2):
                        nc.tensor.matmul(
                            o_h[hl][:, n * 512:(n + 1) * 512],
                            v_sb[:, 15, 2 * m + hl, :],
                            es[hl][:, n * 512:(n + 1) * 512],
                            start=False, stop=True,
                        )
                # pull O' off PSUM (rounded to f32r), reciprocal in place on
                # the denominator row, defer the PE-side normalize
                o_cps, d0s = [], []
                for hl in range(2):
                    o_cp = np_pool.tile([65, 1024], F32R, tag="o_cp",
                                        name=f"o_cp{hl}")
                    with nc.allow_low_precision(reason="f32r matmul input"):
                        nc.vector.tensor_copy(o_cp[:], o_h[hl][:])
                    # reshape the denominator row across all 128 lanes so the
                    # reciprocal runs at full DVE width, then reshape back
                    dsq = d0_pool.tile([128, 8], F32R, tag=f"dsq_{hl}")
                    nc.gpsimd.dma_start(dsq[:], o_cp[64:65, :])
                    with nc.allow_low_precision(reason="f32r matmul input"):
                        nc.vector.reciprocal(dsq[:], dsq[:])
                    d0 = d0_pool.tile([1, 1024], F32R, tag=f"d0_{hl}")
                    nc.gpsimd.dma_start(d0[:], dsq[:])
                    o_cps.append(o_cp)
                    d0s.append(d0)
                pending.append((m, i0, o_cps, d0s))

            for ct in range(8):
                emit_wo_chunk(0, ct)
            while pending:
                emit_normalize()
            for ct in range(8):
                emit_wo_chunk(1, ct)

            d0_pool.release()
            np_pool.release()
            ost_pool.release()
            st_pool.release()
            es_pool.release()

    nc.compile()
    return nc


def kernel(x, Wq, Wk, Wv, Wo, bo):
    x = np.asarray(x, dtype=np.float32)
    Wq = np.asarray(Wq, dtype=np.float32)
    Wk = np.asarray(Wk, dtype=np.float32)
    Wv = np.asarray(Wv, dtype=np.float32)
    Wo = np.asarray(Wo, dtype=np.float32)
    bo = np.asarray(bo, dtype=np.float32)

    if "nc" not in _cache:
        _cache["nc"] = _build()
    nc = _cache["nc"]

    xTs = [np.ascontiguousarray(x[b].T) for b in range(B)]
    in_maps = []
    for core in range(8):
        b, hg = divmod(core, 4)
        sl = slice(hg * HD, (hg + 1) * HD)
        in_maps.append({
            "xT": xTs[b],
            "wq": np.ascontiguousarray(Wq[:, sl]),
            "wk": np.ascontiguousarray(Wk[:, sl]),
            "wv": np.ascontiguousarray(Wv[:, sl]),
            "wo": np.ascontiguousarray(Wo[sl, :]),
        })

    res = run_bass_kernel_spmd(nc, in_maps, core_ids=list(range(8)))
    out = np.empty((B, L, C), dtype=np.float32)
    for b in range(B):
        acc = res.results[4 * b]["outT"]
        for hg in range(1, 4):
            acc = acc + res.results[4 * b + hg]["outT"]
        out[b] = acc.T + bo
    return out
